# revision 11
# baseline (speedup 1.0000x reference)
"""CommNet forward kernel for 8 Trainium2 NeuronCores (Bass/Tile).

Model (B=4096 samples, N=32 agents, O=128 obs, H=256 hidden, A=8, 3 iters):
    h = tanh(obs @ W_enc + b_enc)
    iter0: h = tanh(h @ W_f + b_f + b_C)
    iter1,2: c[b,j] = (sum_i h[b,i] - h[b,j]) / 31
             h = tanh(h @ W_f + c @ W_C + b_f + b_C)
    value = h @ W_v + b_v ; action_mean = h @ W_mean + b_mean

Device strategy:
  * Data-parallel over batch: 512 samples (16384 rows) per core.
  * Feature-major ("transposed") activation layout [feature, row] so every
    matmul keeps the small weight stationary and streams 512 rows per
    instruction. fp16 operands (fp32 PSUM accumulate) for 1 cycle/row and
    pipelined FWL weight loads; fp32-family weight loads serialize with the
    matmul on TRN2 and measured 2.8x slower end-to-end.
  * Comm term is algebraically fused:
        W_f.T @ h.T + W_C.T @ c.T
      = (W_f - W_C/31).T @ h.T + (W_C/31).T @ S_bcast
    where S[f,b] = sum_j h.T[f,(b,j)] is a segmented (per-sample) reduce over
    the 32 contiguous agent columns (DVE), and (W_C/31).T @ S is a 32x smaller
    matmul whose result is broadcast-added into the PSUM accumulator.
  * Heads emit [9, rows] feature-major; the host transposes and applies the
    (zero) head biases while assembling the full output.
"""

import contextlib
import os
import sys

import numpy as np

for _p in ("/opt/trn_rl_repo", "/root/.axon_site/_ro/trn_rl_repo"):
    if os.path.isdir(_p) and _p not in sys.path:
        sys.path.insert(0, _p)

import ml_dtypes  # noqa: E402

import concourse.bacc as bacc  # noqa: E402
import concourse.tile as tile  # noqa: E402
from concourse import mybir  # noqa: E402

B, N_AG, O, H, A = 4096, 32, 128, 256, 8
N_CORES = 8
ROWS = (B // N_CORES) * N_AG  # 16384 rows per core
CHUNK = 512                   # rows per matmul (16 samples)
SAMP = CHUNK // N_AG          # samples per chunk
F32 = mybir.dt.float32
BF16 = mybir.dt.float16
TANH = mybir.ActivationFunctionType.Tanh
X_AXIS = mybir.AxisListType.X
BF16_NP = np.float16


def build(rows=ROWS):
    """Build + schedule the Tile kernel; returns the compiled Bass object."""
    n_chunks = rows // CHUNK
    G = 8     # chunks per stage-major block (software pipelining)
    SUB = 2   # chunks per comm-term sub-batch (latency hiding for the s path)
    nc = bacc.Bacc(
        "TRN2",
        target_bir_lowering=False,
        debug=False,
        enable_asserts=False,
        num_devices=N_CORES,
    )
    obs_t = nc.dram_tensor("obs_t", [O, rows], BF16, kind="ExternalInput").ap()
    w_enc = nc.dram_tensor("w_enc", [128, H], BF16, kind="ExternalInput").ap()
    w_f = nc.dram_tensor("w_f", [128, 2, H], BF16, kind="ExternalInput").ap()
    w_fc = nc.dram_tensor("w_fc", [128, 2, H], BF16, kind="ExternalInput").ap()
    w_c = nc.dram_tensor("w_c", [128, 2, H], BF16, kind="ExternalInput").ap()
    w_hd = nc.dram_tensor("w_hd", [128, 2, 9], BF16, kind="ExternalInput").ap()
    ident = nc.dram_tensor("ident", [128, 128], BF16, kind="ExternalInput").ap()
    b_enc = nc.dram_tensor("b_enc", [128, 2], F32, kind="ExternalInput").ap()
    b_tot = nc.dram_tensor("b_tot", [128, 2], F32, kind="ExternalInput").ap()
    out9 = nc.dram_tensor("out9", [9, rows], F32, kind="ExternalOutput").ap()

    with tile.TileContext(nc, trace_sim=False) as tc, contextlib.ExitStack() as ctx:
        consts = ctx.enter_context(tc.tile_pool(name="consts", bufs=1))
        obs_pool = ctx.enter_context(tc.tile_pool(name="obs", bufs=G + 2))
        hpool = ctx.enter_context(tc.tile_pool(name="h", bufs=G + 2))
        spool = ctx.enter_context(tc.tile_pool(name="s", bufs=3))
        opool = ctx.enter_context(tc.tile_pool(name="o", bufs=6))
        pmain = ctx.enter_context(tc.tile_pool(name="pmain", bufs=5, space="PSUM"))
        psml = ctx.enter_context(tc.tile_pool(name="psml", bufs=1, space="PSUM"))
        phead = ctx.enter_context(tc.tile_pool(name="phead", bufs=2, space="PSUM"))

        w_enc_s = consts.tile([128, H], BF16)
        nc.sync.dma_start(w_enc_s[:], w_enc)
        w_f_s = consts.tile([128, 2, H], BF16)
        nc.sync.dma_start(w_f_s[:], w_f)
        w_fc_s = consts.tile([128, 2, H], BF16)
        nc.sync.dma_start(w_fc_s[:], w_fc)
        w_c_s = consts.tile([128, 2, H], BF16)
        nc.sync.dma_start(w_c_s[:], w_c)
        w_hd_s = consts.tile([128, 2, 9], BF16)
        nc.sync.dma_start(w_hd_s[:], w_hd)
        id_s = consts.tile([128, 128], BF16)
        nc.sync.dma_start(id_s[:], ident)
        b_enc_s = consts.tile([128, 2], F32)
        nc.sync.dma_start(b_enc_s[:], b_enc)
        b_tot_s = consts.tile([128, 2], F32)
        nc.sync.dma_start(b_tot_s[:], b_tot)

        pending_heads = []

        def emit_heads(entries):
            for ci, hk in entries:
                php = phead.tile([9, CHUNK], F32, tag="php")
                for k in range(2):
                    nc.tensor.matmul(
                        php[:], w_hd_s[:, k, :], hk[k][:],
                        start=(k == 0), stop=(k == 1),
                    )
                ot = opool.tile([9, CHUNK], F32, tag="ot")
                nc.scalar.copy(ot[:], php[:])
                nc.sync.dma_start(out9[:, ci * CHUNK:(ci + 1) * CHUNK], ot[:])

        for b0 in range(0, n_chunks, G):
            blk = list(range(b0, min(b0 + G, n_chunks)))
            obs_tiles = {}
            for ci in blk:
                t = obs_pool.tile([128, CHUNK], BF16, tag="obs")
                nc.sync.dma_start(t[:], obs_t[:, ci * CHUNK:(ci + 1) * CHUNK])
                obs_tiles[ci] = t

            # encoder stage (single K chunk)
            h = {}
            for ci in blk:
                h0 = []
                for m in range(2):
                    ps = pmain.tile([128, CHUNK], F32, tag="ps")
                    nc.tensor.matmul(
                        ps[:], w_enc_s[:, m * 128:(m + 1) * 128],
                        obs_tiles[ci][:], start=True, stop=True,
                    )
                    ht = hpool.tile([128, CHUNK], BF16, tag=f"h0m{m}")
                    nc.scalar.activation(
                        ht[:], ps[:], TANH, bias=b_enc_s[:, m:m + 1]
                    )
                    h0.append(ht)
                h[ci] = h0

            # iter0 stage (no comm)
            for ci in blk:
                out = []
                for m in range(2):
                    ps = pmain.tile([128, CHUNK], F32, tag="ps")
                    for k in range(2):
                        nc.tensor.matmul(
                            ps[:], w_f_s[:, k, m * 128:(m + 1) * 128],
                            h[ci][k][:], start=(k == 0), stop=(k == 1),
                        )
                    ht = hpool.tile([128, CHUNK], BF16, tag=f"h1m{m}")
                    nc.scalar.activation(
                        ht[:], ps[:], TANH, bias=b_tot_s[:, m:m + 1]
                    )
                    out.append(ht)
                h[ci] = out

            # previous block's heads: PE/scalar fill while this block's
            # iter pipeline spins up
            emit_heads(pending_heads)
            pending_heads = []

            # iters 1,2 with comm; s path batched per SUB chunks
            for it in (1, 2):
                for s0 in range(0, len(blk), SUB):
                    sub = blk[s0:s0 + SUB]
                    ns = len(sub)
                    # per-sample sums (DVE)
                    sums = []
                    for k in range(2):
                        st = spool.tile([128, ns * SAMP], BF16,
                                        tag=f"sum{it}k{k}")
                        for i, ci in enumerate(sub):
                            with nc.allow_low_precision(reason="fp16 net"):
                                nc.vector.reduce_sum(
                                    st[:, i * SAMP:(i + 1) * SAMP],
                                    h[ci][k][:].rearrange(
                                        "p (s a) -> p s a", a=N_AG
                                    ),
                                    axis=X_AXIS,
                                )
                        sums.append(st)
                    # main matmuls
                    pss = {}
                    for ci in sub:
                        for m in range(2):
                            ps = pmain.tile([128, CHUNK], F32, tag="ps")
                            for k in range(2):
                                nc.tensor.matmul(
                                    ps[:], w_fc_s[:, k, m * 128:(m + 1) * 128],
                                    h[ci][k][:], start=(k == 0),
                                    stop=(k == 1) and (ci % 2 == 1),
                                )
                            pss[(ci, m)] = ps
                    # comm matmul on the sums (N = ns*SAMP)
                    sps = psml.tile([128, 2 * ns * SAMP], F32, tag="sps")
                    for m in range(2):
                        for k in range(2):
                            nc.tensor.matmul(
                                sps[:, m * ns * SAMP:(m + 1) * ns * SAMP],
                                w_c_s[:, k, m * 128:(m + 1) * 128],
                                sums[k][:], start=(k == 0), stop=(k == 1),
                            )
                    s_sb = spool.tile([128, 2 * ns * SAMP], BF16,
                                      tag=f"ssb{it}")
                    nc.vector.tensor_copy(s_sb[:], sps[:])
                    # broadcast-add (even chunks: PE identity-MM, odd: DVE)
                    for ci in sub:
                        i = ci - blk[s0]
                        for m in range(2):
                            ps = pss[(ci, m)]
                            sl = s_sb[:, m * ns * SAMP + i * SAMP:
                                      m * ns * SAMP + (i + 1) * SAMP]
                            if ci % 2 == 0:
                                out_view = ps[:].rearrange(
                                    "p (s a) -> p a s", a=N_AG
                                )
                                rhs_view = sl.unsqueeze(1).broadcast_to(
                                    [128, N_AG, SAMP]
                                )
                                nc.tensor.matmul(
                                    out_view, id_s[:], rhs_view,
                                    start=False, stop=True,
                                    skip_group_check=True,
                                )
                            else:
                                ps3 = ps[:].rearrange(
                                    "p (s a) -> p a s", a=N_AG
                                )
                                bview = sl.unsqueeze(1).broadcast_to(
                                    [128, N_AG, SAMP]
                                )
                                nc.vector.tensor_add(ps3, ps3, bview)
                            ht = hpool.tile([128, CHUNK], BF16,
                                            tag=f"h{it + 1}m{m}")
                            nc.scalar.activation(
                                ht[:], ps[:], TANH, bias=b_tot_s[:, m:m + 1]
                            )
                            if m == 0:
                                h[ci] = [ht]
                            else:
                                h[ci].append(ht)

            # head stage is deferred into the next block (boundary
            # smoothing); the final block's heads flush after the loop
            pending_heads = [(ci, h[ci]) for ci in blk]

        emit_heads(pending_heads)

    nc.compile()
    return nc


def prep_weights(W_enc, b_enc, W_f, b_f, W_C, b_C, W_mean, b_mean, W_v, b_v):
    """Host-side weight layout prep (shared by all cores)."""
    f32 = np.float32

    def kchunk(w):  # [256, X] -> [128, 2, X]  (contraction chunk-major)
        return np.ascontiguousarray(
            w.reshape(2, 128, w.shape[1]).transpose(1, 0, 2)
        ).astype(BF16_NP)

    W_enc = np.asarray(W_enc, f32)
    W_f = np.asarray(W_f, f32)
    W_C = np.asarray(W_C, f32)
    W_Cs = W_C / (N_AG - 1)
    W_head = np.concatenate(
        [np.asarray(W_mean, f32), np.asarray(W_v, f32)], axis=1
    )  # [256, 9]
    bt = (np.asarray(b_f, f32) + np.asarray(b_C, f32)).reshape(2, 128).T
    be = np.asarray(b_enc, f32).reshape(2, 128).T
    return {
        "w_enc": np.ascontiguousarray(W_enc).astype(BF16_NP),
        "w_f": kchunk(W_f),
        "w_fc": kchunk(W_f - W_Cs),
        "w_c": kchunk(W_Cs),
        "w_hd": kchunk(W_head),
        "ident": np.eye(128, dtype=BF16_NP),
        "b_enc": np.ascontiguousarray(be),
        "b_tot": np.ascontiguousarray(bt),
    }


_NC_CACHE = {}


def _get_nc(rows=ROWS):
    if rows not in _NC_CACHE:
        _NC_CACHE[rows] = build(rows)
    return _NC_CACHE[rows]


def _ensure_ntff_hook():
    """The agent image's antenv lacks axon_hooks; synthesize it so
    run_bass_kernel_spmd(trace=True) can profile. Profiling-only path."""
    try:
        from antenv.axon_hooks import get_axon_ntff_profile_hook  # noqa: F401
        return True
    except ImportError:
        pass
    try:
        import types

        import antenv
        from trn_agent_boot.trn_boot import _ntff_profile_via_ctypes

        hook = _ntff_profile_via_ctypes("/opt/axon/libaxon_pjrt.so")
        mod = types.ModuleType("antenv.axon_hooks")
        state = {"hook": hook}
        mod.set_axon_ntff_profile_hook = lambda h: state.__setitem__("hook", h)
        mod.get_axon_ntff_profile_hook = lambda: state["hook"]
        sys.modules["antenv.axon_hooks"] = mod
        antenv.axon_hooks = mod
        return hook is not None
    except Exception:
        return False


def run_device(obs, weights, trace=False):
    """Run the bass kernel on 8 cores. obs: [B, N_AG, O] float32.

    Returns (out9 [B*N_AG, 9] float32, exec_time_ns or None).
    """
    from concourse.bass_utils import run_bass_kernel_spmd

    if trace and not _ensure_ntff_hook():
        trace = False

    nc = _get_nc()
    per = B // N_CORES
    obs = np.asarray(obs, np.float32)
    in_maps = []
    for c in range(N_CORES):
        shard = obs[c * per:(c + 1) * per].reshape(per * N_AG, O)
        obs_tc = np.ascontiguousarray(shard.T).astype(BF16_NP)  # [128, ROWS]
        m = {"obs_t": obs_tc}
        m.update(weights)
        in_maps.append(m)
    res = run_bass_kernel_spmd(
        nc, in_maps, core_ids=list(range(N_CORES)), trace=trace
    )
    outs = [res.results[c]["out9"] for c in range(N_CORES)]  # [9, ROWS] each
    out9 = np.concatenate(outs, axis=1).T  # [B*N_AG, 9]
    return np.ascontiguousarray(out9), res.exec_time_ns


def kernel(**inputs):
    obs = np.asarray(inputs["obs"], np.float32)
    weights = prep_weights(
        inputs["W_enc"], inputs["b_enc"], inputs["W_f"], inputs["b_f"],
        inputs["W_C"], inputs["b_C"], inputs["W_mean"], inputs["b_mean"],
        inputs["W_v"], inputs["b_v"],
    )
    trace = bool(int(os.environ.get("BASS_KERNEL_TRACE", "0")))
    out9, exec_ns = run_device(obs, weights, trace=trace)
    kernel.last_exec_ns = exec_ns

    out9 = out9.reshape(B, N_AG, 9)
    b_mean = np.asarray(inputs["b_mean"], np.float32)
    b_v = np.asarray(inputs["b_v"], np.float32)
    action_mean = out9[:, :, :A] + b_mean
    value = out9[:, :, A:] + b_v
    log_std = np.asarray(inputs["log_std"], np.float32)
    action_log_std = np.broadcast_to(log_std, action_mean.shape).copy()
    action_std = np.exp(action_log_std)
    return (
        action_mean.astype(np.float32),
        action_log_std.astype(np.float32),
        action_std.astype(np.float32),
        np.ascontiguousarray(value, dtype=np.float32),
    )


kernel.last_exec_ns = None


# revision 12
# speedup vs baseline: 1.0172x; 1.0172x over previous
"""CommNet forward kernel for 8 Trainium2 NeuronCores (Bass/Tile).

Model (B=4096 samples, N=32 agents, O=128 obs, H=256 hidden, A=8, 3 iters):
    h = tanh(obs @ W_enc + b_enc)
    iter0: h = tanh(h @ W_f + b_f + b_C)
    iter1,2: c[b,j] = (sum_i h[b,i] - h[b,j]) / 31
             h = tanh(h @ W_f + c @ W_C + b_f + b_C)
    value = h @ W_v + b_v ; action_mean = h @ W_mean + b_mean

Device strategy:
  * Data-parallel over batch: 512 samples (16384 rows) per core.
  * Feature-major ("transposed") activation layout [feature, row] so every
    matmul keeps the small weight stationary and streams 512 rows per
    instruction. fp16 operands (fp32 PSUM accumulate) for 1 cycle/row and
    pipelined FWL weight loads; fp32-family weight loads serialize with the
    matmul on TRN2 and measured 2.8x slower end-to-end.
  * Comm term is algebraically fused:
        W_f.T @ h.T + W_C.T @ c.T
      = (W_f - W_C/31).T @ h.T + (W_C/31).T @ S_bcast
    where S[f,b] = sum_j h.T[f,(b,j)] is a segmented (per-sample) reduce over
    the 32 contiguous agent columns (DVE), and (W_C/31).T @ S is a 32x smaller
    matmul whose result is broadcast-added into the PSUM accumulator.
  * Heads emit [9, rows] feature-major; the host transposes and applies the
    (zero) head biases while assembling the full output.
"""

import contextlib
import os
import sys

import numpy as np

for _p in ("/opt/trn_rl_repo", "/root/.axon_site/_ro/trn_rl_repo"):
    if os.path.isdir(_p) and _p not in sys.path:
        sys.path.insert(0, _p)

import ml_dtypes  # noqa: E402

import concourse.bacc as bacc  # noqa: E402
import concourse.tile as tile  # noqa: E402
from concourse import mybir  # noqa: E402

B, N_AG, O, H, A = 4096, 32, 128, 256, 8
N_CORES = 8
ROWS = (B // N_CORES) * N_AG  # 16384 rows per core
CHUNK = 512                   # rows per matmul (16 samples)
SAMP = CHUNK // N_AG          # samples per chunk
F32 = mybir.dt.float32
BF16 = mybir.dt.float16
TANH = mybir.ActivationFunctionType.Tanh
X_AXIS = mybir.AxisListType.X
BF16_NP = np.float16


def build(rows=ROWS):
    """Build + schedule the Tile kernel; returns the compiled Bass object."""
    n_chunks = rows // CHUNK
    G = 8     # chunks per stage-major block (software pipelining)
    SUB = 2   # chunks per comm-term sub-batch (latency hiding for the s path)
    nc = bacc.Bacc(
        "TRN2",
        target_bir_lowering=False,
        debug=False,
        enable_asserts=False,
        num_devices=N_CORES,
    )
    obs_t = nc.dram_tensor("obs_t", [O, rows], BF16, kind="ExternalInput").ap()
    w_enc = nc.dram_tensor("w_enc", [128, H], BF16, kind="ExternalInput").ap()
    w_f = nc.dram_tensor("w_f", [128, 2, H], BF16, kind="ExternalInput").ap()
    w_fc = nc.dram_tensor("w_fc", [128, 2, H], BF16, kind="ExternalInput").ap()
    w_c = nc.dram_tensor("w_c", [128, 2, H], BF16, kind="ExternalInput").ap()
    w_hd = nc.dram_tensor("w_hd", [128, 2, 9], BF16, kind="ExternalInput").ap()
    ident = nc.dram_tensor("ident", [128, 128], BF16, kind="ExternalInput").ap()
    b_enc = nc.dram_tensor("b_enc", [128, 2], F32, kind="ExternalInput").ap()
    b_tot = nc.dram_tensor("b_tot", [128, 2], F32, kind="ExternalInput").ap()
    out9 = nc.dram_tensor("out9", [9, rows], F32, kind="ExternalOutput").ap()

    with tile.TileContext(nc, trace_sim=False) as tc, contextlib.ExitStack() as ctx:
        consts = ctx.enter_context(tc.tile_pool(name="consts", bufs=1))
        obs_pool = ctx.enter_context(tc.tile_pool(name="obs", bufs=G + 2))
        hpool = ctx.enter_context(tc.tile_pool(name="h", bufs=G + 2))
        spool = ctx.enter_context(tc.tile_pool(name="s", bufs=3))
        opool = ctx.enter_context(tc.tile_pool(name="o", bufs=6))
        pmain = ctx.enter_context(tc.tile_pool(name="pmain", bufs=5, space="PSUM"))
        psml = ctx.enter_context(tc.tile_pool(name="psml", bufs=1, space="PSUM"))
        phead = ctx.enter_context(tc.tile_pool(name="phead", bufs=2, space="PSUM"))

        w_enc_s = consts.tile([128, H], BF16)
        nc.sync.dma_start(w_enc_s[:], w_enc)
        w_f_s = consts.tile([128, 2, H], BF16)
        nc.sync.dma_start(w_f_s[:], w_f)
        w_fc_s = consts.tile([128, 2, H], BF16)
        nc.sync.dma_start(w_fc_s[:], w_fc)
        w_c_s = consts.tile([128, 2, H], BF16)
        nc.sync.dma_start(w_c_s[:], w_c)
        w_hd_s = consts.tile([128, 2, 9], BF16)
        nc.sync.dma_start(w_hd_s[:], w_hd)
        id_s = consts.tile([128, 128], BF16)
        nc.sync.dma_start(id_s[:], ident)
        b_enc_s = consts.tile([128, 2], F32)
        nc.sync.dma_start(b_enc_s[:], b_enc)
        b_tot_s = consts.tile([128, 2], F32)
        nc.sync.dma_start(b_tot_s[:], b_tot)

        pending_heads = []

        def emit_heads(entries):
            for ci, hk in entries:
                php = phead.tile([9, CHUNK], F32, tag="php")
                for k in range(2):
                    nc.tensor.matmul(
                        php[:], w_hd_s[:, k, :], hk[k][:],
                        start=(k == 0), stop=(k == 1),
                    )
                ot = opool.tile([9, CHUNK], F32, tag="ot")
                nc.vector.tensor_copy(ot[:], php[:])
                nc.sync.dma_start(out9[:, ci * CHUNK:(ci + 1) * CHUNK], ot[:])

        for b0 in range(0, n_chunks, G):
            blk = list(range(b0, min(b0 + G, n_chunks)))
            obs_tiles = {}
            for ci in blk:
                t = obs_pool.tile([128, CHUNK], BF16, tag="obs")
                nc.sync.dma_start(t[:], obs_t[:, ci * CHUNK:(ci + 1) * CHUNK])
                obs_tiles[ci] = t

            # encoder stage (single K chunk)
            h = {}
            for ci in blk:
                h0 = []
                for m in range(2):
                    ps = pmain.tile([128, CHUNK], F32, tag="ps")
                    nc.tensor.matmul(
                        ps[:], w_enc_s[:, m * 128:(m + 1) * 128],
                        obs_tiles[ci][:], start=True, stop=True,
                    )
                    ht = hpool.tile([128, CHUNK], BF16, tag=f"h0m{m}")
                    nc.scalar.activation(
                        ht[:], ps[:], TANH, bias=b_enc_s[:, m:m + 1]
                    )
                    h0.append(ht)
                h[ci] = h0

            # iter0 stage (no comm)
            for ci in blk:
                out = []
                for m in range(2):
                    ps = pmain.tile([128, CHUNK], F32, tag="ps")
                    for k in range(2):
                        nc.tensor.matmul(
                            ps[:], w_f_s[:, k, m * 128:(m + 1) * 128],
                            h[ci][k][:], start=(k == 0), stop=(k == 1),
                        )
                    ht = hpool.tile([128, CHUNK], BF16, tag=f"h1m{m}")
                    nc.scalar.activation(
                        ht[:], ps[:], TANH, bias=b_tot_s[:, m:m + 1]
                    )
                    out.append(ht)
                h[ci] = out

            # previous block's heads: PE/scalar fill while this block's
            # iter pipeline spins up
            emit_heads(pending_heads)
            pending_heads = []

            # iters 1,2 with comm; s path batched per SUB chunks
            for it in (1, 2):
                for s0 in range(0, len(blk), SUB):
                    sub = blk[s0:s0 + SUB]
                    ns = len(sub)
                    # per-sample sums (DVE)
                    sums = []
                    for k in range(2):
                        st = spool.tile([128, ns * SAMP], BF16,
                                        tag=f"sum{it}k{k}")
                        for i, ci in enumerate(sub):
                            with nc.allow_low_precision(reason="fp16 net"):
                                nc.vector.reduce_sum(
                                    st[:, i * SAMP:(i + 1) * SAMP],
                                    h[ci][k][:].rearrange(
                                        "p (s a) -> p s a", a=N_AG
                                    ),
                                    axis=X_AXIS,
                                )
                        sums.append(st)
                    # main matmuls
                    pss = {}
                    for ci in sub:
                        for m in range(2):
                            ps = pmain.tile([128, CHUNK], F32, tag="ps")
                            for k in range(2):
                                nc.tensor.matmul(
                                    ps[:], w_fc_s[:, k, m * 128:(m + 1) * 128],
                                    h[ci][k][:], start=(k == 0),
                                    stop=(k == 1) and (ci % 2 == 1),
                                )
                            pss[(ci, m)] = ps
                    # comm matmul on the sums (N = ns*SAMP)
                    sps = psml.tile([128, 2 * ns * SAMP], F32, tag="sps")
                    for m in range(2):
                        for k in range(2):
                            nc.tensor.matmul(
                                sps[:, m * ns * SAMP:(m + 1) * ns * SAMP],
                                w_c_s[:, k, m * 128:(m + 1) * 128],
                                sums[k][:], start=(k == 0), stop=(k == 1),
                            )
                    s_sb = spool.tile([128, 2 * ns * SAMP], BF16,
                                      tag=f"ssb{it}")
                    nc.vector.tensor_copy(s_sb[:], sps[:])
                    # broadcast-add (even chunks: PE identity-MM, odd: DVE)
                    for ci in sub:
                        i = ci - blk[s0]
                        for m in range(2):
                            ps = pss[(ci, m)]
                            sl = s_sb[:, m * ns * SAMP + i * SAMP:
                                      m * ns * SAMP + (i + 1) * SAMP]
                            if ci % 2 == 0:
                                out_view = ps[:].rearrange(
                                    "p (s a) -> p a s", a=N_AG
                                )
                                rhs_view = sl.unsqueeze(1).broadcast_to(
                                    [128, N_AG, SAMP]
                                )
                                nc.tensor.matmul(
                                    out_view, id_s[:], rhs_view,
                                    start=False, stop=True,
                                    skip_group_check=True,
                                )
                            else:
                                ps3 = ps[:].rearrange(
                                    "p (s a) -> p a s", a=N_AG
                                )
                                bview = sl.unsqueeze(1).broadcast_to(
                                    [128, N_AG, SAMP]
                                )
                                nc.vector.tensor_add(ps3, ps3, bview)
                            ht = hpool.tile([128, CHUNK], BF16,
                                            tag=f"h{it + 1}m{m}")
                            nc.scalar.activation(
                                ht[:], ps[:], TANH, bias=b_tot_s[:, m:m + 1]
                            )
                            if m == 0:
                                h[ci] = [ht]
                            else:
                                h[ci].append(ht)

            # head stage is deferred into the next block (boundary
            # smoothing); the final block's heads flush after the loop
            pending_heads = [(ci, h[ci]) for ci in blk]

        emit_heads(pending_heads)

    nc.compile()
    return nc


def prep_weights(W_enc, b_enc, W_f, b_f, W_C, b_C, W_mean, b_mean, W_v, b_v):
    """Host-side weight layout prep (shared by all cores)."""
    f32 = np.float32

    def kchunk(w):  # [256, X] -> [128, 2, X]  (contraction chunk-major)
        return np.ascontiguousarray(
            w.reshape(2, 128, w.shape[1]).transpose(1, 0, 2)
        ).astype(BF16_NP)

    W_enc = np.asarray(W_enc, f32)
    W_f = np.asarray(W_f, f32)
    W_C = np.asarray(W_C, f32)
    W_Cs = W_C / (N_AG - 1)
    W_head = np.concatenate(
        [np.asarray(W_mean, f32), np.asarray(W_v, f32)], axis=1
    )  # [256, 9]
    bt = (np.asarray(b_f, f32) + np.asarray(b_C, f32)).reshape(2, 128).T
    be = np.asarray(b_enc, f32).reshape(2, 128).T
    return {
        "w_enc": np.ascontiguousarray(W_enc).astype(BF16_NP),
        "w_f": kchunk(W_f),
        "w_fc": kchunk(W_f - W_Cs),
        "w_c": kchunk(W_Cs),
        "w_hd": kchunk(W_head),
        "ident": np.eye(128, dtype=BF16_NP),
        "b_enc": np.ascontiguousarray(be),
        "b_tot": np.ascontiguousarray(bt),
    }


_NC_CACHE = {}


def _get_nc(rows=ROWS):
    if rows not in _NC_CACHE:
        _NC_CACHE[rows] = build(rows)
    return _NC_CACHE[rows]


def _ensure_ntff_hook():
    """The agent image's antenv lacks axon_hooks; synthesize it so
    run_bass_kernel_spmd(trace=True) can profile. Profiling-only path."""
    try:
        from antenv.axon_hooks import get_axon_ntff_profile_hook  # noqa: F401
        return True
    except ImportError:
        pass
    try:
        import types

        import antenv
        from trn_agent_boot.trn_boot import _ntff_profile_via_ctypes

        hook = _ntff_profile_via_ctypes("/opt/axon/libaxon_pjrt.so")
        mod = types.ModuleType("antenv.axon_hooks")
        state = {"hook": hook}
        mod.set_axon_ntff_profile_hook = lambda h: state.__setitem__("hook", h)
        mod.get_axon_ntff_profile_hook = lambda: state["hook"]
        sys.modules["antenv.axon_hooks"] = mod
        antenv.axon_hooks = mod
        return hook is not None
    except Exception:
        return False


def run_device(obs, weights, trace=False):
    """Run the bass kernel on 8 cores. obs: [B, N_AG, O] float32.

    Returns (out9 [B*N_AG, 9] float32, exec_time_ns or None).
    """
    from concourse.bass_utils import run_bass_kernel_spmd

    if trace and not _ensure_ntff_hook():
        trace = False

    nc = _get_nc()
    per = B // N_CORES
    obs = np.asarray(obs, np.float32)
    in_maps = []
    for c in range(N_CORES):
        shard = obs[c * per:(c + 1) * per].reshape(per * N_AG, O)
        obs_tc = np.ascontiguousarray(shard.T).astype(BF16_NP)  # [128, ROWS]
        m = {"obs_t": obs_tc}
        m.update(weights)
        in_maps.append(m)
    res = run_bass_kernel_spmd(
        nc, in_maps, core_ids=list(range(N_CORES)), trace=trace
    )
    outs = [res.results[c]["out9"] for c in range(N_CORES)]  # [9, ROWS] each
    out9 = np.concatenate(outs, axis=1).T  # [B*N_AG, 9]
    return np.ascontiguousarray(out9), res.exec_time_ns


def kernel(**inputs):
    obs = np.asarray(inputs["obs"], np.float32)
    weights = prep_weights(
        inputs["W_enc"], inputs["b_enc"], inputs["W_f"], inputs["b_f"],
        inputs["W_C"], inputs["b_C"], inputs["W_mean"], inputs["b_mean"],
        inputs["W_v"], inputs["b_v"],
    )
    trace = bool(int(os.environ.get("BASS_KERNEL_TRACE", "0")))
    out9, exec_ns = run_device(obs, weights, trace=trace)
    kernel.last_exec_ns = exec_ns

    out9 = out9.reshape(B, N_AG, 9)
    b_mean = np.asarray(inputs["b_mean"], np.float32)
    b_v = np.asarray(inputs["b_v"], np.float32)
    action_mean = out9[:, :, :A] + b_mean
    value = out9[:, :, A:] + b_v
    log_std = np.asarray(inputs["log_std"], np.float32)
    action_log_std = np.broadcast_to(log_std, action_mean.shape).copy()
    action_std = np.exp(action_log_std)
    return (
        action_mean.astype(np.float32),
        action_log_std.astype(np.float32),
        action_std.astype(np.float32),
        np.ascontiguousarray(value, dtype=np.float32),
    )


kernel.last_exec_ns = None


# revision 13
# speedup vs baseline: 1.0230x; 1.0057x over previous
"""CommNet forward kernel for 8 Trainium2 NeuronCores (Bass/Tile).

Model (B=4096 samples, N=32 agents, O=128 obs, H=256 hidden, A=8, 3 iters):
    h = tanh(obs @ W_enc + b_enc)
    iter0: h = tanh(h @ W_f + b_f + b_C)
    iter1,2: c[b,j] = (sum_i h[b,i] - h[b,j]) / 31
             h = tanh(h @ W_f + c @ W_C + b_f + b_C)
    value = h @ W_v + b_v ; action_mean = h @ W_mean + b_mean

Device strategy:
  * Data-parallel over batch: 512 samples (16384 rows) per core.
  * Feature-major ("transposed") activation layout [feature, row] so every
    matmul keeps the small weight stationary and streams 512 rows per
    instruction. fp16 operands (fp32 PSUM accumulate) for 1 cycle/row and
    pipelined FWL weight loads; fp32-family weight loads serialize with the
    matmul on TRN2 and measured 2.8x slower end-to-end.
  * Comm term is algebraically fused:
        W_f.T @ h.T + W_C.T @ c.T
      = (W_f - W_C/31).T @ h.T + (W_C/31).T @ S_bcast
    where S[f,b] = sum_j h.T[f,(b,j)] is a segmented (per-sample) reduce over
    the 32 contiguous agent columns (DVE), and (W_C/31).T @ S is a 32x smaller
    matmul whose result is broadcast-added into the PSUM accumulator.
  * Heads emit [9, rows] feature-major; the host transposes and applies the
    (zero) head biases while assembling the full output.
"""

import contextlib
import os
import sys

import numpy as np

for _p in ("/opt/trn_rl_repo", "/root/.axon_site/_ro/trn_rl_repo"):
    if os.path.isdir(_p) and _p not in sys.path:
        sys.path.insert(0, _p)

import ml_dtypes  # noqa: E402

import concourse.bacc as bacc  # noqa: E402
import concourse.tile as tile  # noqa: E402
from concourse import mybir  # noqa: E402

B, N_AG, O, H, A = 4096, 32, 128, 256, 8
N_CORES = 8
ROWS = (B // N_CORES) * N_AG  # 16384 rows per core
CHUNK = 512                   # rows per matmul (16 samples)
SAMP = CHUNK // N_AG          # samples per chunk
F32 = mybir.dt.float32
BF16 = mybir.dt.float16
TANH = mybir.ActivationFunctionType.Tanh
X_AXIS = mybir.AxisListType.X
BF16_NP = np.float16


def build(rows=ROWS):
    """Build + schedule the Tile kernel; returns the compiled Bass object."""
    n_chunks = rows // CHUNK
    G = 8     # chunks per stage-major block (software pipelining)
    SUB = 2   # chunks per comm-term sub-batch (latency hiding for the s path)
    nc = bacc.Bacc(
        "TRN2",
        target_bir_lowering=False,
        debug=False,
        enable_asserts=False,
        num_devices=N_CORES,
    )
    obs_t = nc.dram_tensor("obs_t", [O, rows], BF16, kind="ExternalInput").ap()
    w_enc = nc.dram_tensor("w_enc", [128, H], BF16, kind="ExternalInput").ap()
    w_f = nc.dram_tensor("w_f", [128, 2, H], BF16, kind="ExternalInput").ap()
    w_fc = nc.dram_tensor("w_fc", [128, 2, H], BF16, kind="ExternalInput").ap()
    w_c = nc.dram_tensor("w_c", [128, 2, H], BF16, kind="ExternalInput").ap()
    w_hd = nc.dram_tensor("w_hd", [128, 2, 9], BF16, kind="ExternalInput").ap()
    ident = nc.dram_tensor("ident", [128, 128], BF16, kind="ExternalInput").ap()
    b_enc = nc.dram_tensor("b_enc", [128, 2], F32, kind="ExternalInput").ap()
    b_tot = nc.dram_tensor("b_tot", [128, 2], F32, kind="ExternalInput").ap()
    out9 = nc.dram_tensor("out9", [9, rows], F32, kind="ExternalOutput").ap()

    with tile.TileContext(nc, trace_sim=False) as tc, contextlib.ExitStack() as ctx:
        consts = ctx.enter_context(tc.tile_pool(name="consts", bufs=1))
        obs_pool = ctx.enter_context(tc.tile_pool(name="obs", bufs=G + 2))
        hpool = ctx.enter_context(tc.tile_pool(name="h", bufs=G + 2))
        spool = ctx.enter_context(tc.tile_pool(name="s", bufs=3))
        opool = ctx.enter_context(tc.tile_pool(name="o", bufs=6))
        pmain = ctx.enter_context(tc.tile_pool(name="pmain", bufs=6, space="PSUM"))
        psml = ctx.enter_context(tc.tile_pool(name="psml", bufs=1, space="PSUM"))
        phead = ctx.enter_context(tc.tile_pool(name="phead", bufs=1, space="PSUM"))

        w_enc_s = consts.tile([128, H], BF16)
        nc.sync.dma_start(w_enc_s[:], w_enc)
        w_f_s = consts.tile([128, 2, H], BF16)
        nc.sync.dma_start(w_f_s[:], w_f)
        w_fc_s = consts.tile([128, 2, H], BF16)
        nc.sync.dma_start(w_fc_s[:], w_fc)
        w_c_s = consts.tile([128, 2, H], BF16)
        nc.sync.dma_start(w_c_s[:], w_c)
        w_hd_s = consts.tile([128, 2, 9], BF16)
        nc.sync.dma_start(w_hd_s[:], w_hd)
        id_s = consts.tile([128, 128], BF16)
        nc.sync.dma_start(id_s[:], ident)
        b_enc_s = consts.tile([128, 2], F32)
        nc.sync.dma_start(b_enc_s[:], b_enc)
        b_tot_s = consts.tile([128, 2], F32)
        nc.sync.dma_start(b_tot_s[:], b_tot)

        pending_heads = []

        def emit_heads(entries):
            for ci, hk in entries:
                php = phead.tile([9, CHUNK], F32, tag="php")
                for k in range(2):
                    nc.tensor.matmul(
                        php[:], w_hd_s[:, k, :], hk[k][:],
                        start=(k == 0), stop=(k == 1),
                    )
                ot = opool.tile([9, CHUNK], F32, tag="ot")
                nc.vector.tensor_copy(ot[:], php[:])
                nc.sync.dma_start(out9[:, ci * CHUNK:(ci + 1) * CHUNK], ot[:])

        for b0 in range(0, n_chunks, G):
            blk = list(range(b0, min(b0 + G, n_chunks)))
            obs_tiles = {}
            for ci in blk:
                t = obs_pool.tile([128, CHUNK], BF16, tag="obs")
                nc.sync.dma_start(t[:], obs_t[:, ci * CHUNK:(ci + 1) * CHUNK])
                obs_tiles[ci] = t

            # encoder stage (single K chunk)
            h = {}
            for ci in blk:
                h0 = []
                for m in range(2):
                    ps = pmain.tile([128, CHUNK], F32, tag="ps")
                    nc.tensor.matmul(
                        ps[:], w_enc_s[:, m * 128:(m + 1) * 128],
                        obs_tiles[ci][:], start=True, stop=True,
                    )
                    ht = hpool.tile([128, CHUNK], BF16, tag=f"h0m{m}")
                    nc.scalar.activation(
                        ht[:], ps[:], TANH, bias=b_enc_s[:, m:m + 1]
                    )
                    h0.append(ht)
                h[ci] = h0

            # iter0 stage (no comm)
            for ci in blk:
                out = []
                for m in range(2):
                    ps = pmain.tile([128, CHUNK], F32, tag="ps")
                    for k in range(2):
                        nc.tensor.matmul(
                            ps[:], w_f_s[:, k, m * 128:(m + 1) * 128],
                            h[ci][k][:], start=(k == 0), stop=(k == 1),
                        )
                    ht = hpool.tile([128, CHUNK], BF16, tag=f"h1m{m}")
                    nc.scalar.activation(
                        ht[:], ps[:], TANH, bias=b_tot_s[:, m:m + 1]
                    )
                    out.append(ht)
                h[ci] = out

            # previous block's heads: PE/scalar fill while this block's
            # iter pipeline spins up
            emit_heads(pending_heads)
            pending_heads = []

            # iters 1,2 with comm; s path batched per SUB chunks
            for it in (1, 2):
                for s0 in range(0, len(blk), SUB):
                    sub = blk[s0:s0 + SUB]
                    ns = len(sub)
                    # per-sample sums (DVE)
                    sums = []
                    for k in range(2):
                        st = spool.tile([128, ns * SAMP], BF16,
                                        tag=f"sum{it}k{k}")
                        for i, ci in enumerate(sub):
                            with nc.allow_low_precision(reason="fp16 net"):
                                nc.vector.reduce_sum(
                                    st[:, i * SAMP:(i + 1) * SAMP],
                                    h[ci][k][:].rearrange(
                                        "p (s a) -> p s a", a=N_AG
                                    ),
                                    axis=X_AXIS,
                                )
                        sums.append(st)
                    # main matmuls
                    pss = {}
                    for ci in sub:
                        for m in range(2):
                            ps = pmain.tile([128, CHUNK], F32, tag="ps")
                            for k in range(2):
                                nc.tensor.matmul(
                                    ps[:], w_fc_s[:, k, m * 128:(m + 1) * 128],
                                    h[ci][k][:], start=(k == 0),
                                    stop=(k == 1) and (ci % 2 == 1),
                                )
                            pss[(ci, m)] = ps
                    # comm matmul on the sums (N = ns*SAMP)
                    sps = psml.tile([128, 2 * ns * SAMP], F32, tag="sps")
                    for m in range(2):
                        for k in range(2):
                            nc.tensor.matmul(
                                sps[:, m * ns * SAMP:(m + 1) * ns * SAMP],
                                w_c_s[:, k, m * 128:(m + 1) * 128],
                                sums[k][:], start=(k == 0), stop=(k == 1),
                            )
                    s_sb = spool.tile([128, 2 * ns * SAMP], BF16,
                                      tag=f"ssb{it}")
                    nc.vector.tensor_copy(s_sb[:], sps[:])
                    # broadcast-add (even chunks: PE identity-MM, odd: DVE)
                    for ci in sub:
                        i = ci - blk[s0]
                        for m in range(2):
                            ps = pss[(ci, m)]
                            sl = s_sb[:, m * ns * SAMP + i * SAMP:
                                      m * ns * SAMP + (i + 1) * SAMP]
                            if ci % 2 == 0:
                                out_view = ps[:].rearrange(
                                    "p (s a) -> p a s", a=N_AG
                                )
                                rhs_view = sl.unsqueeze(1).broadcast_to(
                                    [128, N_AG, SAMP]
                                )
                                nc.tensor.matmul(
                                    out_view, id_s[:], rhs_view,
                                    start=False, stop=True,
                                    skip_group_check=True,
                                )
                            else:
                                ps3 = ps[:].rearrange(
                                    "p (s a) -> p a s", a=N_AG
                                )
                                bview = sl.unsqueeze(1).broadcast_to(
                                    [128, N_AG, SAMP]
                                )
                                nc.vector.tensor_add(ps3, ps3, bview)
                            ht = hpool.tile([128, CHUNK], BF16,
                                            tag=f"h{it + 1}m{m}")
                            nc.scalar.activation(
                                ht[:], ps[:], TANH, bias=b_tot_s[:, m:m + 1]
                            )
                            if m == 0:
                                h[ci] = [ht]
                            else:
                                h[ci].append(ht)

            # head stage is deferred into the next block (boundary
            # smoothing); the final block's heads flush after the loop
            pending_heads = [(ci, h[ci]) for ci in blk]

        emit_heads(pending_heads)

    nc.compile()
    return nc


def prep_weights(W_enc, b_enc, W_f, b_f, W_C, b_C, W_mean, b_mean, W_v, b_v):
    """Host-side weight layout prep (shared by all cores)."""
    f32 = np.float32

    def kchunk(w):  # [256, X] -> [128, 2, X]  (contraction chunk-major)
        return np.ascontiguousarray(
            w.reshape(2, 128, w.shape[1]).transpose(1, 0, 2)
        ).astype(BF16_NP)

    W_enc = np.asarray(W_enc, f32)
    W_f = np.asarray(W_f, f32)
    W_C = np.asarray(W_C, f32)
    W_Cs = W_C / (N_AG - 1)
    W_head = np.concatenate(
        [np.asarray(W_mean, f32), np.asarray(W_v, f32)], axis=1
    )  # [256, 9]
    bt = (np.asarray(b_f, f32) + np.asarray(b_C, f32)).reshape(2, 128).T
    be = np.asarray(b_enc, f32).reshape(2, 128).T
    return {
        "w_enc": np.ascontiguousarray(W_enc).astype(BF16_NP),
        "w_f": kchunk(W_f),
        "w_fc": kchunk(W_f - W_Cs),
        "w_c": kchunk(W_Cs),
        "w_hd": kchunk(W_head),
        "ident": np.eye(128, dtype=BF16_NP),
        "b_enc": np.ascontiguousarray(be),
        "b_tot": np.ascontiguousarray(bt),
    }


_NC_CACHE = {}


def _get_nc(rows=ROWS):
    if rows not in _NC_CACHE:
        _NC_CACHE[rows] = build(rows)
    return _NC_CACHE[rows]


def _ensure_ntff_hook():
    """The agent image's antenv lacks axon_hooks; synthesize it so
    run_bass_kernel_spmd(trace=True) can profile. Profiling-only path."""
    try:
        from antenv.axon_hooks import get_axon_ntff_profile_hook  # noqa: F401
        return True
    except ImportError:
        pass
    try:
        import types

        import antenv
        from trn_agent_boot.trn_boot import _ntff_profile_via_ctypes

        hook = _ntff_profile_via_ctypes("/opt/axon/libaxon_pjrt.so")
        mod = types.ModuleType("antenv.axon_hooks")
        state = {"hook": hook}
        mod.set_axon_ntff_profile_hook = lambda h: state.__setitem__("hook", h)
        mod.get_axon_ntff_profile_hook = lambda: state["hook"]
        sys.modules["antenv.axon_hooks"] = mod
        antenv.axon_hooks = mod
        return hook is not None
    except Exception:
        return False


def run_device(obs, weights, trace=False):
    """Run the bass kernel on 8 cores. obs: [B, N_AG, O] float32.

    Returns (out9 [B*N_AG, 9] float32, exec_time_ns or None).
    """
    from concourse.bass_utils import run_bass_kernel_spmd

    if trace and not _ensure_ntff_hook():
        trace = False

    nc = _get_nc()
    per = B // N_CORES
    obs = np.asarray(obs, np.float32)
    in_maps = []
    for c in range(N_CORES):
        shard = obs[c * per:(c + 1) * per].reshape(per * N_AG, O)
        obs_tc = np.ascontiguousarray(shard.T).astype(BF16_NP)  # [128, ROWS]
        m = {"obs_t": obs_tc}
        m.update(weights)
        in_maps.append(m)
    res = run_bass_kernel_spmd(
        nc, in_maps, core_ids=list(range(N_CORES)), trace=trace
    )
    outs = [res.results[c]["out9"] for c in range(N_CORES)]  # [9, ROWS] each
    out9 = np.concatenate(outs, axis=1).T  # [B*N_AG, 9]
    return np.ascontiguousarray(out9), res.exec_time_ns


def kernel(**inputs):
    obs = np.asarray(inputs["obs"], np.float32)
    weights = prep_weights(
        inputs["W_enc"], inputs["b_enc"], inputs["W_f"], inputs["b_f"],
        inputs["W_C"], inputs["b_C"], inputs["W_mean"], inputs["b_mean"],
        inputs["W_v"], inputs["b_v"],
    )
    trace = bool(int(os.environ.get("BASS_KERNEL_TRACE", "0")))
    out9, exec_ns = run_device(obs, weights, trace=trace)
    kernel.last_exec_ns = exec_ns

    out9 = out9.reshape(B, N_AG, 9)
    b_mean = np.asarray(inputs["b_mean"], np.float32)
    b_v = np.asarray(inputs["b_v"], np.float32)
    action_mean = out9[:, :, :A] + b_mean
    value = out9[:, :, A:] + b_v
    log_std = np.asarray(inputs["log_std"], np.float32)
    action_log_std = np.broadcast_to(log_std, action_mean.shape).copy()
    action_std = np.exp(action_log_std)
    return (
        action_mean.astype(np.float32),
        action_log_std.astype(np.float32),
        action_std.astype(np.float32),
        np.ascontiguousarray(value, dtype=np.float32),
    )


kernel.last_exec_ns = None
